# revision 1
# baseline (speedup 1.0000x reference)
"""Trainium2 Bass kernel for nn_Discriminator_67027259621837.

MLP: [x,y] -> tanh(. @ W0 + b0) -> 20x[ sin(. @ Wm + bm); softplus(. @ Wm + bm) ]
      -> . @ Wl + bl,  N = 2,000,000 rows, width 40, weight-shared mid layers.

Strategy (8 NeuronCores, pure data parallel over the batch):
  * Each core gets R = 250,000 contiguous rows; weights replicated.
  * On-chip layout: activations transposed, [120 partitions, C cols] fp16.
    Three overlapping row-groups of C = (R+2)//3 = 83,334 rows are packed
    block-diagonally (3 x 40 features = 120 partitions), so every matmul /
    activation instruction uses 120 of 128 lanes.  The two duplicated rows
    compute bitwise-identically, so the overlapping output stores are benign.
  * Weights packed block-diagonal [120, 120] fp16; PSUM accumulates fp32.
  * Layer-major loop over the whole chunk: ScalarE activation-table sets for
    Sin vs Exp/Ln differ, so alternating per-tile would cost a ~2.7us table
    load per switch; layer-major pays it only once per layer.
  * softplus(x) = Ln(Exp(x) + 1): two ACT passes, both functions live in the
    natural_log_exp_and_others table (single load per softplus layer).
  * The big activation buffer A is updated in place per layer; Tile's
    range-granular dependency tracking orders matmul-reads vs ACT-writes.
"""

import os

import numpy as np

N_FULL = 2_000_000
NCORES = 8
R = N_FULL // NCORES  # rows per core
WIDTH = 40
NMID = 40
SB = 2048    # superblock columns (4 PSUM banks), ping-ponged
NSB = 41     # superblocks per layer; NSB*SB >= C
MMN = 512    # matmul moving-dim (one PSUM bank of fp32)

_NC_CACHE = None
LAST_RESULTS = None


def _build(R, SB, NSB, MMN):
    from contextlib import ExitStack

    import concourse.bacc as bacc
    import concourse.bass as bass
    import concourse.tile as tile
    from concourse import mybir

    AF = mybir.ActivationFunctionType
    dt = mybir.dt

    C = (R + 2) // 3
    assert 3 * C - 2 == R, R
    CPAD = NSB * SB
    assert CPAD >= C and SB % MMN == 0
    Q = SB // MMN
    STEP = C - 1  # row stride between the three groups

    nc = bacc.Bacc("TRN2", target_bir_lowering=False)

    x = nc.dram_tensor("x", [R, 1], dt.float32, kind="ExternalInput")
    y = nc.dram_tensor("y", [R, 1], dt.float32, kind="ExternalInput")
    W0 = nc.dram_tensor("W0", [2, WIDTH], dt.float32, kind="ExternalInput")
    b0 = nc.dram_tensor("b0", [WIDTH], dt.float32, kind="ExternalInput")
    Wm = nc.dram_tensor("Wm", [WIDTH, WIDTH], dt.float32, kind="ExternalInput")
    bm = nc.dram_tensor("bm", [WIDTH], dt.float32, kind="ExternalInput")
    Wl = nc.dram_tensor("Wl", [WIDTH, 1], dt.float32, kind="ExternalInput")
    bl = nc.dram_tensor("bl", [1], dt.float32, kind="ExternalInput")
    out = nc.dram_tensor("out", [R, 1], dt.float32, kind="ExternalOutput")

    P3 = 3 * WIDTH  # 120

    with tile.TileContext(nc) as tc, ExitStack() as ctx:
        const = ctx.enter_context(tc.tile_pool(name="const", bufs=1))
        abuf_p = ctx.enter_context(tc.tile_pool(name="abuf", bufs=1))
        xy_p = ctx.enter_context(tc.tile_pool(name="xy", bufs=2))
        et_p = ctx.enter_context(tc.tile_pool(name="etmp", bufs=2))
        st_p = ctx.enter_context(tc.tile_pool(name="stage", bufs=1))
        ps_p = ctx.enter_context(tc.tile_pool(name="psum", bufs=2, space="PSUM"))

        # ---------------- constants -----------------
        # W0_3 [6, 120] fp32: rows 0-2 = x-weights for groups A,B,C;
        # rows 3-5 = y-weights. Column block 40k belongs to group k.
        W0_3 = const.tile([6, P3], dt.float32)
        nc.vector.memset(W0_3[:], 0.0)
        for k in range(3):
            nc.sync.dma_start(W0_3[k : k + 1, k * WIDTH : (k + 1) * WIDTH],
                              W0[0:1, :])
            nc.sync.dma_start(W0_3[3 + k : 4 + k, k * WIDTH : (k + 1) * WIDTH],
                              W0[1:2, :])

        Wm_sb = const.tile([WIDTH, WIDTH], dt.float32)
        nc.sync.dma_start(Wm_sb[:], Wm[:, :])
        Wm16 = const.tile([WIDTH, WIDTH], dt.float16)
        nc.vector.tensor_copy(Wm16[:], Wm_sb[:])
        Wm3 = const.tile([P3, P3], dt.float16)
        nc.vector.memset(Wm3[:], 0.0)
        for k in range(3):
            nc.sync.dma_start(
                Wm3[k * WIDTH : (k + 1) * WIDTH, k * WIDTH : (k + 1) * WIDTH],
                Wm16[:])

        Wl_sb = const.tile([WIDTH, 1], dt.float32)
        nc.sync.dma_start(Wl_sb[:], Wl[:, :])
        Wl16 = const.tile([WIDTH, 1], dt.float16)
        nc.vector.tensor_copy(Wl16[:], Wl_sb[:])
        Wl3 = const.tile([P3, 3], dt.float16)
        nc.vector.memset(Wl3[:], 0.0)
        for k in range(3):
            nc.sync.dma_start(Wl3[k * WIDTH : (k + 1) * WIDTH, k : k + 1],
                              Wl16[:])

        b0_3 = const.tile([P3, 1], dt.float32)
        bm_3 = const.tile([P3, 1], dt.float32)
        for k in range(3):
            nc.sync.dma_start(b0_3[k * WIDTH : (k + 1) * WIDTH, 0:1],
                              bass.AP(b0, 0, [[1, WIDTH], [1, 1]]))
            nc.sync.dma_start(bm_3[k * WIDTH : (k + 1) * WIDTH, 0:1],
                              bass.AP(bm, 0, [[1, WIDTH], [1, 1]]))
        bl_3 = const.tile([3, 1], dt.float32)
        for k in range(3):
            nc.sync.dma_start(bl_3[k : k + 1, 0:1],
                              bass.AP(bl, 0, [[1, 1], [1, 1]]))

        # Activation buffer: whole per-core chunk, fp16, updated in place.
        A = abuf_p.tile([P3, CPAD], dt.float16)

        # ---------------- layer 0: tanh(xy @ W0 + b0) -----------------
        for s in range(NSB):
            n = max(0, min(SB, C - s * SB))  # valid cols this superblock
            xy = xy_p.tile([6, SB], dt.float32)
            if n > 0:
                nc.sync.dma_start(xy[0:3, 0:n],
                                  bass.AP(x, s * SB, [[STEP, 3], [1, n]]))
                nc.sync.dma_start(xy[3:6, 0:n],
                                  bass.AP(y, s * SB, [[STEP, 3], [1, n]]))
            ps = ps_p.tile([128, SB], dt.float32)
            for q in range(Q):
                nc.tensor.matmul(ps[0:P3, q * MMN : (q + 1) * MMN],
                                 W0_3[:],
                                 xy[:, q * MMN : (q + 1) * MMN],
                                 start=True, stop=True)
            nc.scalar.activation(A[:, s * SB : (s + 1) * SB], ps[0:P3, :],
                                 AF.Tanh, bias=b0_3[:])

        # ---------------- middle layers -----------------
        for li in range(1, NMID + 1):
            is_sin = (li % 2 == 1)
            for s in range(NSB):
                cs = slice(s * SB, (s + 1) * SB)
                ps = ps_p.tile([128, SB], dt.float32)
                for q in range(Q):
                    c0 = s * SB + q * MMN
                    nc.tensor.matmul(ps[0:P3, q * MMN : (q + 1) * MMN],
                                     Wm3[:],
                                     A[:, c0 : c0 + MMN],
                                     start=True, stop=True)
                if is_sin:
                    nc.scalar.activation(A[:, cs], ps[0:P3, :],
                                         AF.Sin, bias=bm_3[:])
                else:
                    et = et_p.tile([P3, SB], dt.float16)
                    nc.scalar.activation(et[:], ps[0:P3, :],
                                         AF.Exp, bias=bm_3[:])
                    nc.scalar.activation(A[:, cs], et[:], AF.Ln, bias=1.0)

        # ---------------- final layer: A @ Wl + bl -----------------
        for s in range(NSB):
            n = max(0, min(SB, C - s * SB))
            ps = ps_p.tile([128, SB], dt.float32)
            for q in range(Q):
                c0 = s * SB + q * MMN
                nc.tensor.matmul(ps[0:3, q * MMN : (q + 1) * MMN],
                                 Wl3[:],
                                 A[:, c0 : c0 + MMN],
                                 start=True, stop=True)
            st = st_p.tile([3, SB], dt.float32)
            nc.vector.tensor_scalar_add(st[:], ps[0:3, :], bl_3[:])
            if n > 0:
                nc.sync.dma_start(bass.AP(out, s * SB, [[STEP, 3], [1, n]]),
                                  st[0:3, 0:n])

    nc.compile()
    return nc


def _get_nc():
    global _NC_CACHE
    if _NC_CACHE is None:
        _NC_CACHE = _build(R, SB, NSB, MMN)
    return _NC_CACHE


def kernel(x, y, W0, b0, Wm, bm, Wl, bl):
    global LAST_RESULTS
    from concourse.bass_utils import run_bass_kernel_spmd

    f32 = lambda a: np.ascontiguousarray(np.asarray(a, dtype=np.float32))
    x, y = f32(x), f32(y)
    W0, b0, Wm, bm, Wl, bl = f32(W0), f32(b0), f32(Wm), f32(bm), f32(Wl), f32(bl)

    nc = _get_nc()
    in_maps = []
    for i in range(NCORES):
        sl = slice(i * R, (i + 1) * R)
        in_maps.append({
            "x": x[sl], "y": y[sl],
            "W0": W0, "b0": b0, "Wm": Wm, "bm": bm, "Wl": Wl, "bl": bl,
        })
    kw = {}
    if os.environ.get("BASS_KERNEL_TRACE"):
        kw["trace"] = True
    res = run_bass_kernel_spmd(nc, in_maps, core_ids=list(range(NCORES)), **kw)
    LAST_RESULTS = res
    return np.concatenate([r["out"] for r in res.results], axis=0)
